# revision 36
# baseline (speedup 1.0000x reference)
"""Trainium2 Bass kernel for nn_CLM_26594437496868 (co-attention + conv/BN/leakyrelu).

Reference computation (b=4, c=64, h=w=64, hw=4096):
  EL = W_lin @ E                       # [c, hw] per sample
  A[n, m] = sum_c EL[c, n] Q[c, m]     # [hw, hw]
  query_c[c, n]    = sum_m Q[c, m] exp(A[n, m]) / sum_m exp(A[n, m])
  exemplar_c[c, n] = sum_m E[c, m] exp(A[m, n]) / sum_m exp(A[m, n])
  out_x = query_c + exemplar_c + E + Q
  y = conv3x3(out_x, W_conv); y = BN(y) * gamma + beta; leaky_relu(y, 0.1)

Sharding: 8 cores = 4 samples x 2 image-halves.  Per core the attention
"l" axis is the padded half [64 phantom | 64 halo | 2048 own | 64 halo |
64 phantom] = 2304 cols; "m" is the full 4096, host-permuted so the own
half comes first.  Orientation 0 (rows of A) runs first with l-major PV
accumulation ([l,65] psum, ones-column denominators); orientation 1
(cols of A) reuses exp of the diagonal block A[own, own] from phase A
via DMA-engine blocked transposes instead of recomputing score+exp.
Exp work is split between the ACT engine (table exp) and the DVE
(Schraudolph int16 bit-trick exp, bitcast to bf16).  The normalized sum
is DMA-transposed back to channel-major for the conv; BN batch stats go
through one tiny AllGather.
"""
import sys
if "/opt/trn_rl_repo" not in sys.path:
    sys.path.append("/opt/trn_rl_repo")

import numpy as np

import concourse.bass as bass
import concourse.bacc as bacc
import concourse.tile as tile
from concourse import mybir
from concourse import bass_utils

N_CORES = 8
C = 64
HW = 4096
W_IMG = 64
LCH = 18                  # l chunks of 128
L = LCH * 128             # 2304
M_CHUNKS = 32
L_BLOCKS = [(0, 4), (4, 4), (8, 4), (12, 4), (16, 2)]   # (chunk0, nchunks)
BN_EPS = 1e-5
LEAKY = 0.1
DIAG = True               # reuse exp(A[own,own]) via transpose in phase B

# Schraudolph exp in bf16-bit domain: bits = round(A_S*x + B_S)
A_S = 128.0 / float(np.log(2.0))
B_S = 16256.0 - 5.6

BF16 = mybir.dt.bfloat16
F32 = mybir.dt.float32
I16 = mybir.dt.int16
NPBF16 = mybir.dt.np(BF16)

_COMPILED = None


def _build_program():
    nc = bacc.Bacc("TRN2", target_bir_lowering=False, debug=False,
                   enable_asserts=True, num_devices=N_CORES)

    # ---- I/O ----
    # pack: [wt | eh | xq | qh | xe | eqh | wconv]
    PACKW = C + L + HW + L + HW + L + 9 * C
    CRIT1 = C + L            # wt + eh  (gates el_l block 0)
    CRIT2 = CRIT1 + HW       # + xq    (gates orientation-0 scores)
    d_pack = nc.dram_tensor("pack", [C, PACKW], BF16, kind="ExternalInput").ap()
    d_xq = nc.dram_tensor("xq", [C, HW], BF16, kind="ExternalInput").ap()
    d_xe = nc.dram_tensor("xe", [C, HW], BF16, kind="ExternalInput").ap()
    d_mask = nc.dram_tensor("maskl", [128, LCH], BF16, kind="ExternalInput").ap()
    d_gb = nc.dram_tensor("gb", [C, 2], F32, kind="ExternalInput").ap()
    d_out = nc.dram_tensor("out", [C, 2048], F32, kind="ExternalOutput").ap()

    from contextlib import ExitStack
    with tile.TileContext(nc) as tc, ExitStack() as ctx:
        consts = ctx.enter_context(tc.tile_pool(name="consts", bufs=1))
        big = ctx.enter_context(tc.tile_pool(name="big", bufs=1))
        expp = ctx.enter_context(tc.tile_pool(name="expp", bufs=8))
        extp = ctx.enter_context(tc.tile_pool(name="extp", bufs=8))
        smalls = ctx.enter_context(tc.tile_pool(name="smalls", bufs=3))
        dram = ctx.enter_context(tc.tile_pool(name="dram", bufs=1, space="DRAM"))
        ps_sp = ctx.enter_context(tc.tile_pool(name="ps_sp", bufs=6, space="PSUM"))
        ps_pv = ctx.enter_context(tc.tile_pool(name="ps_pv", bufs=2, space="PSUM"))

        # ---- input DMAs (criticality-split) ----
        pack_sb = big.tile([C, PACKW], BF16)
        CRIT0 = C + 512
        nc.sync.dma_start(out=pack_sb[:, 0:CRIT0], in_=d_pack[:, 0:CRIT0])
        nc.sync.dma_start(out=pack_sb[:, CRIT0:CRIT1], in_=d_pack[:, CRIT0:CRIT1])
        CRITH = CRIT1 + 2048
        nc.sync.dma_start(out=pack_sb[:, CRIT1:CRITH], in_=d_pack[:, CRIT1:CRITH])
        nc.sync.dma_start(out=pack_sb[:, CRITH:CRIT2], in_=d_pack[:, CRITH:CRIT2])
        o0 = 0
        wt_sb = pack_sb[:, o0:o0 + C]; o0 += C
        eh_sb = pack_sb[:, o0:o0 + L]; o0 += L
        xq_sb = pack_sb[:, o0:o0 + HW]; o0 += HW
        qh_sb = pack_sb[:, o0:o0 + L]; o0 += L
        xe_sb = pack_sb[:, o0:o0 + HW]; o0 += HW
        eqh_sb = pack_sb[:, o0:o0 + L]; o0 += L
        wconv_sb = pack_sb[:, o0:o0 + 9 * C].rearrange("p (t o) -> p t o", t=9)
        o0 += 9 * C

        # PV weights: [Q^T | 1], [E^T | 1]  via DMA transpose + strided copy
        qt_sb = big.tile([128, M_CHUNKS, C + 1], BF16)
        et_sb = big.tile([128, M_CHUNKS, C + 1], BF16)
        qtd = big.tile([128, M_CHUNKS, C], BF16)
        etd = big.tile([128, M_CHUNKS, C], BF16)
        nc.sync.dma_start_transpose(out=qtd[:], in_=d_xq[:])
        nc.sync.dma_start(out=qt_sb[:, :, 0:C], in_=qtd[:])
        nc.vector.memset(qt_sb[:, :, C:C + 1], 1.0)
        nc.sync.dma_start(out=pack_sb[:, CRIT2:], in_=d_pack[:, CRIT2:])
        nc.sync.dma_start_transpose(out=etd[:], in_=d_xe[:])
        nc.sync.dma_start(out=et_sb[:, :, 0:C], in_=etd[:])
        nc.vector.memset(et_sb[:, :, C:C + 1], 1.0)

        mask_sb = consts.tile([128, LCH], BF16)
        nc.sync.dma_start(out=mask_sb[:], in_=d_mask[:])
        gb_sb = consts.tile([C, 2], F32)
        nc.sync.dma_start(out=gb_sb[:], in_=d_gb[:])
        gamma_sb = gb_sb[:, 0:1]
        beta_sb = gb_sb[:, 1:2]

        alpha_sb = consts.tile([C, 1], F32)
        nc.vector.memset(alpha_sb[:], LEAKY)
        eps_sb = consts.tile([C, 1], F32)
        nc.vector.memset(eps_sb[:], BN_EPS)
        warm_sb = consts.tile([C, 1], F32)
        nc.scalar.activation(out=warm_sb[:], in_=eps_sb[:],
                             func=mybir.ActivationFunctionType.Ln)
        nc.scalar.activation(out=warm_sb[:], in_=eps_sb[:],
                             func=mybir.ActivationFunctionType.Exp)
        # warm the PE p-state during the input DMA wait
        wps = ps_sp.tile([128, 1, 512], F32, tag="sp")
        for w in range(10):
            nc.tensor.matmul(wps[0:C, 0, :], wt_sb[:],
                             pack_sb[:, 64:64 + 512],
                             start=(w == 0), stop=(w == 9))

        # ---- persistent buffers ----
        el_l = big.tile([C, L], BF16)          # EL over l cols (o0 rhs)
        elf = big.tile([C, HW], BF16)          # EL over m cols (o1 lhsT)
        if DIAG:
            G = big.tile([128, 16, L], BF16)   # exp(T1) for m-chunks 0..15
        z0s = big.tile([128, LCH, C], BF16)    # normalized o0 output, l-major
        zbuf = [big.tile([128, 4, 128], BF16, name=f"zbuf{i}")
                for i in range(2)]
        for zb in zbuf:
            nc.vector.memset(zb[:, :, C:128], 0.0)
        ztb = [big.tile([128, 4, 128], BF16, name=f"ztb{i}")
               for i in range(2)]
        xpad = big.tile([C, 36, 66], BF16)     # conv input, rows = l rows
        nc.vector.memset(xpad[:, :, 0:1], 0.0)
        nc.vector.memset(xpad[:, :, 65:66], 0.0)
        y_sb = big.tile([C, 2048], F32)
        st = smalls.tile([C, 4, 6], F32, tag="st")

        # ---- EL matmuls (queued into strip flow via generator) ----
        def el_jobs():
            for b in range(5):                 # el_l blocks (gate o0 scores)
                w = 512 if b < 4 else 256
                yield (el_l, L_BLOCKS[b][0] * 128, w, eh_sb)
            for b in range(8):                 # elf blocks (phase B lhsT)
                yield (elf, b * 512, 512, xe_sb)
        _el_iter = iter(el_jobs())

        def emit_el(n=1):
            for _ in range(n):
                job = next(_el_iter, None)
                if job is None:
                    return
                dst, off, w, src = job
                ps = ps_sp.tile([128, 1, 512], F32, tag="sp")
                nc.tensor.matmul(ps[0:C, 0, 0:w], wt_sb[:],
                                 src[:, off:off + w], start=True, stop=True)
                nc.vector.tensor_copy(dst[:, off:off + w], ps[0:C, 0, 0:w])

        # ---- exp helpers ----
        def emit_exp(sp_ap, dst_ap, use_dve):
            if use_dve:
                nc.vector.tensor_scalar(out=dst_ap.bitcast(I16), in0=sp_ap,
                                        scalar1=A_S, scalar2=B_S,
                                        op0=mybir.AluOpType.mult,
                                        op1=mybir.AluOpType.add)
            else:
                nc.scalar.activation(out=dst_ap, in_=sp_ap,
                                     func=mybir.ActivationFunctionType.Exp)

        # ---- ext transpose prefetch (phase B, DIAG) ----
        def emit_ext(bi):
            c0, nch = L_BLOCKS[bi]
            d = {}
            for k in range(nch):
                kg = c0 + k
                if 1 <= kg <= 16:
                    ex_t = extp.tile([128, 16, 128], BF16, tag="ext")
                    nc.sync.dma_start_transpose(
                        out=ex_t[:], in_=G[:, kg - 1, 128:2176])
                    d[k] = ex_t
            return d

        # ---- one orientation-block of attention ----
        def run_block(bi, orient, exts=None, hooks=()):
            c0, nch = L_BLOCKS[bi]
            nbl = nch * 128
            lhs = xq_sb if orient == 0 else elf
            rhs = el_l if orient == 0 else qh_sb
            pvw = qt_sb if orient == 0 else et_sb
            pv = ps_pv.tile([128, 4, C + 1], F32, tag="pv")
            hooks = list(hooks)

            started = [False]

            def sflag():
                if not started[0]:
                    started[0] = True
                    return True
                return False

            diag1 = orient == 1 and DIAG
            if diag1:
                strips = [(16 + i, 1) for i in range(16)]
                edge = [k for k in range(nch) if not (1 <= c0 + k <= 16)]
            else:
                strips = [(i, 1) for i in range(32)]
                edge = []

            # edge strips: fresh scores for m 0..15 on the halo l-chunk(s)
            for k in edge:
                for si in range(4):           # 4 strips x 4 m-chunks x 128 l
                    spt = ps_sp.tile([128, 1, 512], F32, tag="sp")
                    sp4 = spt.rearrange("p a (b c) -> p (a b) c", c=128)
                    for u in range(4):
                        j = 4 * si + u
                        nc.tensor.matmul(
                            sp4[:, u, :], lhs[:, 128 * j:128 * j + 128],
                            rhs[:, (c0 + k) * 128:(c0 + k) * 128 + 128],
                            start=True, stop=True)
                    ex = expp.tile([128, 1, 512], BF16, tag="ex")
                    ex4 = ex.rearrange("p a (b c) -> p (a b) c", c=128)
                    emit_exp(sp4[:, 0:2, :], ex4[:, 0:2, :], False)
                    emit_exp(sp4[:, 2:4, :], ex4[:, 2:4, :], True)
                    for u in range(4):
                        j = 4 * si + u
                        nc.tensor.matmul(pv[:, k, :], ex4[:, u, :],
                                         pvw[:, j, :],
                                         start=sflag(), stop=False)

            # main strips (software-pipelined: exp+PV fire 2 strips behind)
            prevq = []
            for si, (j0, ns) in enumerate(strips):
                spt = ps_sp.tile([128, 1, 512], F32, tag="sp")
                for u in range(ns):
                    j = j0 + u
                    nc.tensor.matmul(spt[:, u, 0:nbl],
                                     lhs[:, 128 * j:128 * j + 128],
                                     rhs[:, c0 * 128:c0 * 128 + nbl],
                                     start=True, stop=True)

                if orient == 0 and DIAG and j0 < 16:
                    dst2 = G[:, j0:j0 + 1, c0 * 128:c0 * 128 + nbl]
                    dsl = lambda u, k, j0=j0, c0=c0: \
                        G[:, j0 + u, (c0 + k) * 128:(c0 + k + 1) * 128]
                else:
                    ex = expp.tile([128, 1, 512], BF16, tag="ex")
                    dst2 = ex[:, 0:1, 0:nbl]
                    dsl = lambda u, k, ex=ex: ex[:, u, 128 * k:128 * (k + 1)]

                def fire(spt=spt, dsl=dsl, dst2=dst2, j0=j0, ns=ns,
                         si=si):
                    emit_exp(spt[:, 0:ns, 0:nbl], dst2[:, 0:ns], (si % 7) % 2 == 1)
                    for u in range(ns):
                        j = j0 + u
                        for k in range(nch):
                            nc.tensor.matmul(
                                pv[:, k, :], dsl(u, k), pvw[:, j, :],
                                start=sflag(),
                                stop=(j == M_CHUNKS - 1))
                prevq.append(fire)
                if len(prevq) > 4:
                    prevq.pop(0)()
                if diag1:
                    for k in range(nch):
                        if k in exts:
                            nc.tensor.matmul(
                                pv[:, k, :], exts[k][:, si, :],
                                pvw[:, si, :], start=sflag(), stop=False)
                if hooks:
                    h = hooks.pop(0)
                    if h is not None:
                        h()
                if orient == 0 and bi < 2:
                    emit_el(1)
            for f in prevq:
                f()
            for h in hooks:
                h()

            # ---- deferred normalize closure ----
            def finish():
                rd = smalls.tile([128, 4], F32, tag="rd")
                nc.vector.reciprocal(rd[:, 0:nch], pv[:, 0:nch, C])
                rdm = smalls.tile([128, 4], F32, tag="rdm")
                nc.vector.tensor_mul(rdm[:, 0:nch], rd[:, 0:nch],
                                     mask_sb[:, c0:c0 + nch])
                rdb = rdm[:, 0:nch].rearrange(
                    "p (a b) -> p a b", b=1).broadcast_to([128, nch, C])
                if orient == 0:
                    nc.vector.tensor_mul(z0s[:, c0:c0 + nch, :],
                                         pv[:, 0:nch, 0:C], rdb)
                else:
                    zb = zbuf[bi % 2]
                    zt = ztb[bi % 2]
                    z1 = smalls.tile([128, 4, C], BF16, tag="z1")
                    nc.vector.tensor_mul(z1[:, 0:nch, :],
                                         pv[:, 0:nch, 0:C], rdb)
                    nc.vector.tensor_add(zb[:, 0:nch, 0:C], z1[:, 0:nch, :],
                                         z0s[:, c0:c0 + nch, :])
                    nc.sync.dma_start_transpose(
                        out=zt[:, 0:nch, :],
                        in_=zb[:, 0:nch, :].rearrange("p a b -> p (a b)"))
                    nc.vector.tensor_add(
                        xpad[:, 2 * c0:2 * c0 + 2 * nch, 1:65],
                        zt[0:C, 0:nch, :].rearrange(
                            "p a (r w) -> p (a r) w", w=W_IMG),
                        eqh_sb[:, c0 * 128:c0 * 128 + nbl].rearrange(
                            "p (r w) -> p r w", w=W_IMG))
            return finish

        # ---- conv row-block ----
        def emit_conv(rb):
            ypt = ps_sp.tile([128, 1, 512], F32, tag="sp")
            yp = ypt[0:C, 0, :]
            for tap in range(9):
                dy, dx = tap // 3, tap % 3
                nc.tensor.matmul(
                    yp,
                    wconv_sb[:, tap, :],
                    xpad[:, 8 * rb + 1 + dy:8 * rb + 9 + dy, dx:dx + 64],
                    start=(tap == 0), stop=(tap == 8))
            nc.vector.bn_stats(out=st[:, rb, :], in_=yp)
            nc.vector.tensor_copy(y_sb[:, rb * 512:(rb + 1) * 512], yp)

        # ---- phase A (orientation 0) ----
        emit_el(1)            # el_l block 0 gates the first scores
        fin = None
        for bi in range(5):
            fin = run_block(bi, 0, hooks=[fin] if fin else [])
        fin()
        emit_el(13)           # any stragglers

        # ---- phase B (orientation 1) + interleaved conv ----
        ext_q = []
        if DIAG:
            ext_q = [emit_ext(0)]
        fin = None
        for bi in range(5):
            if DIAG and bi + 1 < 5:
                ext_q.append(emit_ext(bi + 1))
            hooks = []
            if fin is not None:
                hooks.extend([None, fin])
            if bi >= 2:
                hooks.extend([None] * 7)
                hooks.append(lambda rb=bi - 2: emit_conv(rb))
            fin = run_block(bi, 1, exts=ext_q.pop(0) if DIAG else None,
                            hooks=hooks)
        fin()
        emit_conv(2)
        emit_conv(3)

        mv = smalls.tile([C, 2], F32, tag="mv")
        nc.vector.bn_aggr(out=mv[:], in_=st[:])

        # ---- BN stats AllGather + local reduce ----
        ccs = smalls.tile([C, 2], F32, tag="ccs")
        nc.vector.tensor_copy(ccs[:, 0:1], mv[:, 0:1])
        nc.vector.scalar_tensor_tensor(
            out=ccs[:, 1:2], in0=mv[:, 0:1], scalar=mv[:, 0:1],
            in1=mv[:, 1:2], op0=mybir.AluOpType.mult,
            op1=mybir.AluOpType.add)
        cc_in = dram.tile([C, 2], F32)
        cc_out = dram.tile([N_CORES, C, 2], F32, addr_space="Shared")
        nc.sync.dma_start(out=cc_in[:], in_=ccs[:])
        nc.gpsimd.collective_compute(
            "AllGather", mybir.AluOpType.bypass,
            replica_groups=[list(range(N_CORES))],
            ins=[cc_in.opt()], outs=[cc_out.opt()])
        gath = smalls.tile([C, 2, N_CORES], F32, tag="gath")
        nc.sync.dma_start(out=gath[:],
                          in_=cc_out[:].rearrange("r c v -> c v r"))
        red = smalls.tile([C, 2], F32, tag="red")
        nc.vector.tensor_reduce(red[:], gath[:], axis=mybir.AxisListType.X,
                                op=mybir.AluOpType.add)

        # mu = red0/8 ; var = red1/8 - mu^2 ; rstd = exp(-0.5*ln(var+eps))
        nc.vector.tensor_scalar_mul(red[:], red[:], 1.0 / N_CORES)
        mu = red[:, 0:1]
        var = smalls.tile([C, 1], F32, tag="var")
        mu2 = smalls.tile([C, 1], F32, tag="mu2")
        nc.vector.tensor_mul(mu2[:], mu, mu)
        nc.vector.tensor_sub(var[:], red[:, 1:2], mu2[:])
        lnv = smalls.tile([C, 1], F32, tag="lnv")
        nc.scalar.activation(out=lnv[:], in_=var[:],
                             func=mybir.ActivationFunctionType.Ln,
                             bias=eps_sb[:])
        rstd = smalls.tile([C, 1], F32, tag="rstd")
        nc.scalar.activation(out=rstd[:], in_=lnv[:],
                             func=mybir.ActivationFunctionType.Exp, scale=-0.5)
        scale_f = smalls.tile([C, 1], F32, tag="scale_f")
        bias_f = smalls.tile([C, 1], F32, tag="bias_f")
        nc.vector.tensor_mul(scale_f[:], gamma_sb[:], rstd[:])
        nc.vector.tensor_mul(bias_f[:], mu, scale_f[:])
        nc.vector.tensor_sub(bias_f[:], beta_sb[:], bias_f[:])

        # ---- BN + leaky relu (Prelu), write out in 4 chunks ----
        for hb in range(4):
            sl = slice(hb * 512, (hb + 1) * 512)
            osb = smalls.tile([C, 512], F32, tag="osb")
            nc.scalar.activation(out=osb[:], in_=y_sb[:, sl],
                                 func=mybir.ActivationFunctionType.Prelu,
                                 bias=bias_f[:], scale=scale_f[:],
                                 alpha=alpha_sb[:])
            nc.sync.dma_start(out=d_out[:, sl], in_=osb[:])

    nc.compile()
    return nc


def _get_program():
    global _COMPILED
    if _COMPILED is None:
        _COMPILED = _build_program()
    return _COMPILED


def _make_in_maps(exemplar, query, W_lin, W_conv, gamma, beta):
    E = np.asarray(exemplar, dtype=np.float32).reshape(4, C, HW)
    Q = np.asarray(query, dtype=np.float32).reshape(4, C, HW)
    wt = np.ascontiguousarray(np.asarray(W_lin, np.float32).T)
    wconv = np.ascontiguousarray(
        np.asarray(W_conv, np.float32).transpose(1, 2, 3, 0).reshape(C, 9 * C))
    g = np.asarray(gamma, np.float32).reshape(C, 1)
    b = np.asarray(beta, np.float32).reshape(C, 1)

    in_maps = []
    for k in range(N_CORES):
        s, h = divmod(k, 2)
        own = slice(2048 * h, 2048 * h + 2048)
        oth = slice(2048 * (1 - h), 2048 * (1 - h) + 2048)
        xq_p = np.concatenate([Q[s][:, own], Q[s][:, oth]], axis=1)
        xe_p = np.concatenate([E[s][:, own], E[s][:, oth]], axis=1)

        # l layout: 36 rows; l-row r -> sample row 32h - 2 + r (r=1..34)
        ehl = np.zeros((C, L), np.float32)
        qhl = np.zeros((C, L), np.float32)
        maskl = np.zeros((128, LCH), np.float32)
        for r in range(36):
            sr = 32 * h - 2 + r
            if 1 <= r <= 34 and 0 <= sr < 64:
                ehl[:, 64 * r:64 * r + 64] = E[s][:, 64 * sr:64 * sr + 64]
                qhl[:, 64 * r:64 * r + 64] = Q[s][:, 64 * sr:64 * sr + 64]
                kk, half = divmod(r, 2)
                maskl[64 * half:64 * half + 64, kk] = 1.0
        pack = np.concatenate([
            wt, ehl, xq_p, qhl, xe_p, (ehl + qhl), wconv], axis=1
        ).astype(NPBF16)
        in_maps.append({
            "pack": np.ascontiguousarray(pack),
            "xq": np.ascontiguousarray(xq_p.astype(NPBF16)),
            "xe": np.ascontiguousarray(xe_p.astype(NPBF16)),
            "maskl": np.ascontiguousarray(maskl.astype(NPBF16)),
            "gb": np.ascontiguousarray(np.concatenate([g, b], axis=1)),
        })
    return in_maps


def kernel(exemplar, query, W_lin, W_conv, gamma, beta):
    nc = _get_program()
    in_maps = _make_in_maps(exemplar, query, W_lin, W_conv, gamma, beta)
    res = bass_utils.run_bass_kernel_spmd(
        nc, in_maps, core_ids=list(range(N_CORES)), trace=False)
    out = np.empty((4, C, 64, 64), np.float32)
    for k in range(N_CORES):
        s, h = divmod(k, 2)
        out[s, :, 32 * h:32 * h + 32, :] = \
            res.results[k]["out"].reshape(C, 32, 64)
    return out


# revision 37
# speedup vs baseline: 1.0174x; 1.0174x over previous
"""Trainium2 Bass kernel for nn_CLM_26594437496868 (co-attention + conv/BN/leakyrelu).

Reference computation (b=4, c=64, h=w=64, hw=4096):
  EL = W_lin @ E                       # [c, hw] per sample
  A[n, m] = sum_c EL[c, n] Q[c, m]     # [hw, hw]
  query_c[c, n]    = sum_m Q[c, m] exp(A[n, m]) / sum_m exp(A[n, m])
  exemplar_c[c, n] = sum_m E[c, m] exp(A[m, n]) / sum_m exp(A[m, n])
  out_x = query_c + exemplar_c + E + Q
  y = conv3x3(out_x, W_conv); y = BN(y) * gamma + beta; leaky_relu(y, 0.1)

Sharding: 8 cores = 4 samples x 2 image-halves.  Per core the attention
"l" axis is the padded half [64 phantom | 64 halo | 2048 own | 64 halo |
64 phantom] = 2304 cols; "m" is the full 4096, host-permuted so the own
half comes first.  Orientation 0 (rows of A) runs first with l-major PV
accumulation ([l,65] psum, ones-column denominators); orientation 1
(cols of A) reuses exp of the diagonal block A[own, own] from phase A
via DMA-engine blocked transposes instead of recomputing score+exp.
Exp work is split between the ACT engine (table exp) and the DVE
(Schraudolph int16 bit-trick exp, bitcast to bf16).  The normalized sum
is DMA-transposed back to channel-major for the conv; BN batch stats go
through one tiny AllGather.
"""
import sys
if "/opt/trn_rl_repo" not in sys.path:
    sys.path.append("/opt/trn_rl_repo")

import numpy as np

import concourse.bass as bass
import concourse.bacc as bacc
import concourse.tile as tile
from concourse import mybir
from concourse import bass_utils

N_CORES = 8
C = 64
HW = 4096
W_IMG = 64
LCH = 18                  # l chunks of 128
L = LCH * 128             # 2304
M_CHUNKS = 32
L_BLOCKS = [(0, 4), (4, 4), (8, 4), (12, 4), (16, 2)]   # (chunk0, nchunks)
BN_EPS = 1e-5
LEAKY = 0.1
DIAG = True               # reuse exp(A[own,own]) via transpose in phase B

# Schraudolph exp in bf16-bit domain: bits = round(A_S*x + B_S)
A_S = 128.0 / float(np.log(2.0))
B_S = 16256.0 - 5.6

BF16 = mybir.dt.bfloat16
F32 = mybir.dt.float32
I16 = mybir.dt.int16
NPBF16 = mybir.dt.np(BF16)

_COMPILED = None


def _build_program():
    nc = bacc.Bacc("TRN2", target_bir_lowering=False, debug=False,
                   enable_asserts=True, num_devices=N_CORES)

    # ---- I/O ----
    # pack: [wt | eh | xq | qh | xe | eqh | wconv]
    PACKW = C + L + HW + L + HW + L + 9 * C
    CRIT1 = C + L            # wt + eh  (gates el_l block 0)
    CRIT2 = CRIT1 + HW       # + xq    (gates orientation-0 scores)
    d_pack = nc.dram_tensor("pack", [C, PACKW], BF16, kind="ExternalInput").ap()
    d_xq = nc.dram_tensor("xq", [C, HW], BF16, kind="ExternalInput").ap()
    d_xe = nc.dram_tensor("xe", [C, HW], BF16, kind="ExternalInput").ap()
    d_mask = nc.dram_tensor("maskl", [128, LCH], BF16, kind="ExternalInput").ap()
    d_gb = nc.dram_tensor("gb", [C, 2], F32, kind="ExternalInput").ap()
    d_out = nc.dram_tensor("out", [C, 2048], F32, kind="ExternalOutput").ap()

    from contextlib import ExitStack
    with tile.TileContext(nc) as tc, ExitStack() as ctx:
        consts = ctx.enter_context(tc.tile_pool(name="consts", bufs=1))
        big = ctx.enter_context(tc.tile_pool(name="big", bufs=1))
        expp = ctx.enter_context(tc.tile_pool(name="expp", bufs=8))
        extp = ctx.enter_context(tc.tile_pool(name="extp", bufs=8))
        smalls = ctx.enter_context(tc.tile_pool(name="smalls", bufs=3))
        dram = ctx.enter_context(tc.tile_pool(name="dram", bufs=1, space="DRAM"))
        ps_sp = ctx.enter_context(tc.tile_pool(name="ps_sp", bufs=6, space="PSUM"))
        ps_pv = ctx.enter_context(tc.tile_pool(name="ps_pv", bufs=2, space="PSUM"))

        # ---- input DMAs (criticality-split) ----
        pack_sb = big.tile([C, PACKW], BF16)
        CRIT0 = C + 512
        nc.sync.dma_start(out=pack_sb[:, 0:CRIT0], in_=d_pack[:, 0:CRIT0])
        nc.sync.dma_start(out=pack_sb[:, CRIT0:CRIT1], in_=d_pack[:, CRIT0:CRIT1])
        CRITH = CRIT1 + 2048
        nc.sync.dma_start(out=pack_sb[:, CRIT1:CRITH], in_=d_pack[:, CRIT1:CRITH])
        nc.sync.dma_start(out=pack_sb[:, CRITH:CRIT2], in_=d_pack[:, CRITH:CRIT2])
        o0 = 0
        wt_sb = pack_sb[:, o0:o0 + C]; o0 += C
        eh_sb = pack_sb[:, o0:o0 + L]; o0 += L
        xq_sb = pack_sb[:, o0:o0 + HW]; o0 += HW
        qh_sb = pack_sb[:, o0:o0 + L]; o0 += L
        xe_sb = pack_sb[:, o0:o0 + HW]; o0 += HW
        eqh_sb = pack_sb[:, o0:o0 + L]; o0 += L
        wconv_sb = pack_sb[:, o0:o0 + 9 * C].rearrange("p (t o) -> p t o", t=9)
        o0 += 9 * C

        # PV weights: [Q^T | 1], [E^T | 1]  via DMA transpose + strided copy
        qt_sb = big.tile([128, M_CHUNKS, C + 1], BF16)
        et_sb = big.tile([128, M_CHUNKS, C + 1], BF16)
        qtd = big.tile([128, M_CHUNKS, C], BF16)
        etd = big.tile([128, M_CHUNKS, C], BF16)
        nc.sync.dma_start_transpose(out=qtd[:], in_=d_xq[:])
        nc.sync.dma_start(out=qt_sb[:, :, 0:C], in_=qtd[:])
        nc.vector.memset(qt_sb[:, :, C:C + 1], 1.0)
        nc.sync.dma_start(out=pack_sb[:, CRIT2:], in_=d_pack[:, CRIT2:])
        nc.sync.dma_start_transpose(out=etd[:], in_=d_xe[:])
        nc.sync.dma_start(out=et_sb[:, :, 0:C], in_=etd[:])
        nc.vector.memset(et_sb[:, :, C:C + 1], 1.0)

        mask_sb = consts.tile([128, LCH], BF16)
        nc.sync.dma_start(out=mask_sb[:], in_=d_mask[:])
        gb_sb = consts.tile([C, 2], F32)
        nc.sync.dma_start(out=gb_sb[:], in_=d_gb[:])
        gamma_sb = gb_sb[:, 0:1]
        beta_sb = gb_sb[:, 1:2]

        alpha_sb = consts.tile([C, 1], F32)
        nc.vector.memset(alpha_sb[:], LEAKY)
        eps_sb = consts.tile([C, 1], F32)
        nc.vector.memset(eps_sb[:], BN_EPS)
        warm_sb = consts.tile([C, 1], F32)
        nc.scalar.activation(out=warm_sb[:], in_=eps_sb[:],
                             func=mybir.ActivationFunctionType.Ln)
        nc.scalar.activation(out=warm_sb[:], in_=eps_sb[:],
                             func=mybir.ActivationFunctionType.Exp)
        # warm the PE p-state during the input DMA wait
        wps = ps_sp.tile([128, 1, 512], F32, tag="sp")
        for w in range(10):
            nc.tensor.matmul(wps[0:C, 0, :], wt_sb[:],
                             pack_sb[:, 64:64 + 512],
                             start=(w == 0), stop=(w == 9))

        # ---- persistent buffers ----
        el_l = big.tile([C, L], BF16)          # EL over l cols (o0 rhs)
        elf = big.tile([C, HW], BF16)          # EL over m cols (o1 lhsT)
        if DIAG:
            G = big.tile([128, 16, L], BF16)   # exp(T1) for m-chunks 0..15
        z0s = big.tile([128, LCH, C], BF16)    # normalized o0 output, l-major
        zbuf = [big.tile([128, 4, 128], BF16, name=f"zbuf{i}")
                for i in range(2)]
        for zb in zbuf:
            nc.vector.memset(zb[:, :, C:128], 0.0)
        ztb = [big.tile([128, 4, 128], BF16, name=f"ztb{i}")
               for i in range(2)]
        xpad = big.tile([C, 36, 66], BF16)     # conv input, rows = l rows
        nc.vector.memset(xpad[:, :, 0:1], 0.0)
        nc.vector.memset(xpad[:, :, 65:66], 0.0)
        y_sb = big.tile([C, 2048], F32)
        st = smalls.tile([C, 4, 6], F32, tag="st")

        # ---- EL matmuls (queued into strip flow via generator) ----
        def el_jobs():
            for b in range(5):                 # el_l blocks (gate o0 scores)
                w = 512 if b < 4 else 256
                yield (el_l, L_BLOCKS[b][0] * 128, w, eh_sb)
            for b in range(8):                 # elf blocks (phase B lhsT)
                yield (elf, b * 512, 512, xe_sb)
        _el_iter = iter(el_jobs())

        def emit_el(n=1):
            for _ in range(n):
                job = next(_el_iter, None)
                if job is None:
                    return
                dst, off, w, src = job
                ps = ps_sp.tile([128, 1, 512], F32, tag="sp")
                nc.tensor.matmul(ps[0:C, 0, 0:w], wt_sb[:],
                                 src[:, off:off + w], start=True, stop=True)
                nc.vector.tensor_copy(dst[:, off:off + w], ps[0:C, 0, 0:w])

        # ---- exp helpers ----
        def emit_exp(sp_ap, dst_ap, use_dve):
            if use_dve:
                nc.vector.tensor_scalar(out=dst_ap.bitcast(I16), in0=sp_ap,
                                        scalar1=A_S, scalar2=B_S,
                                        op0=mybir.AluOpType.mult,
                                        op1=mybir.AluOpType.add)
            else:
                nc.scalar.activation(out=dst_ap, in_=sp_ap,
                                     func=mybir.ActivationFunctionType.Exp)

        # ---- ext transpose prefetch (phase B, DIAG) ----
        def emit_ext(bi):
            c0, nch = L_BLOCKS[bi]
            d = {}
            for k in range(nch):
                kg = c0 + k
                if 1 <= kg <= 16:
                    ex_t = extp.tile([128, 16, 128], BF16, tag="ext")
                    nc.sync.dma_start_transpose(
                        out=ex_t[:], in_=G[:, kg - 1, 128:2176])
                    d[k] = ex_t
            return d

        # ---- one orientation-block of attention ----
        def run_block(bi, orient, exts=None, hooks=()):
            c0, nch = L_BLOCKS[bi]
            nbl = nch * 128
            lhs = xq_sb if orient == 0 else elf
            rhs = el_l if orient == 0 else qh_sb
            pvw = qt_sb if orient == 0 else et_sb
            pv = ps_pv.tile([128, 4, C + 1], F32, tag="pv")
            hooks = list(hooks)

            started = [False]

            def sflag():
                if not started[0]:
                    started[0] = True
                    return True
                return False

            diag1 = orient == 1 and DIAG
            if diag1:
                strips = [(16 + i, 1) for i in range(16)]
                edge = [k for k in range(nch) if not (1 <= c0 + k <= 16)]
            else:
                strips = [(i, 1) for i in range(32)]
                edge = []

            # edge strips: fresh scores for m 0..15 on the halo l-chunk(s)
            for k in edge:
                for si in range(4):           # 4 strips x 4 m-chunks x 128 l
                    spt = ps_sp.tile([128, 1, 512], F32, tag="sp")
                    sp4 = spt.rearrange("p a (b c) -> p (a b) c", c=128)
                    for u in range(4):
                        j = 4 * si + u
                        nc.tensor.matmul(
                            sp4[:, u, :], lhs[:, 128 * j:128 * j + 128],
                            rhs[:, (c0 + k) * 128:(c0 + k) * 128 + 128],
                            start=True, stop=True)
                    ex = expp.tile([128, 1, 512], BF16, tag="ex")
                    ex4 = ex.rearrange("p a (b c) -> p (a b) c", c=128)
                    emit_exp(sp4[:, 0:2, :], ex4[:, 0:2, :], False)
                    emit_exp(sp4[:, 2:4, :], ex4[:, 2:4, :], True)
                    for u in range(4):
                        j = 4 * si + u
                        nc.tensor.matmul(pv[:, k, :], ex4[:, u, :],
                                         pvw[:, j, :],
                                         start=sflag(), stop=False)

            # main strips (software-pipelined: exp+PV fire 2 strips behind)
            prevq = []
            for si, (j0, ns) in enumerate(strips):
                spt = ps_sp.tile([128, 1, 512], F32, tag="sp")
                for u in range(ns):
                    j = j0 + u
                    nc.tensor.matmul(spt[:, u, 0:nbl],
                                     lhs[:, 128 * j:128 * j + 128],
                                     rhs[:, c0 * 128:c0 * 128 + nbl],
                                     start=True, stop=True)

                if orient == 0 and DIAG and j0 < 16:
                    dst2 = G[:, j0:j0 + 1, c0 * 128:c0 * 128 + nbl]
                    dsl = lambda u, k, j0=j0, c0=c0: \
                        G[:, j0 + u, (c0 + k) * 128:(c0 + k + 1) * 128]
                else:
                    ex = expp.tile([128, 1, 512], BF16, tag="ex")
                    dst2 = ex[:, 0:1, 0:nbl]
                    dsl = lambda u, k, ex=ex: ex[:, u, 128 * k:128 * (k + 1)]

                def fire(spt=spt, dsl=dsl, dst2=dst2, j0=j0, ns=ns,
                         si=si):
                    emit_exp(spt[:, 0:ns, 0:nbl], dst2[:, 0:ns], (si % 7) % 2 == 1)
                    for u in range(ns):
                        j = j0 + u
                        for k in range(nch):
                            nc.tensor.matmul(
                                pv[:, k, :], dsl(u, k), pvw[:, j, :],
                                start=sflag(),
                                stop=(j == M_CHUNKS - 1))
                prevq.append(fire)
                if len(prevq) > 4:
                    prevq.pop(0)()
                if diag1:
                    for k in range(nch):
                        if k in exts:
                            nc.tensor.matmul(
                                pv[:, k, :], exts[k][:, si, :],
                                pvw[:, si, :], start=sflag(), stop=False)
                if hooks:
                    h = hooks.pop(0)
                    if h is not None:
                        h()
                if orient == 0 and bi < 2:
                    emit_el(1)
            for f in prevq:
                f()
            for h in hooks:
                h()

            # ---- deferred normalize closure ----
            def finish():
                rd = smalls.tile([128, 4], F32, tag="rd")
                nc.vector.reciprocal(rd[:, 0:nch], pv[:, 0:nch, C])
                rdm = smalls.tile([128, 4], F32, tag="rdm")
                nc.vector.tensor_mul(rdm[:, 0:nch], rd[:, 0:nch],
                                     mask_sb[:, c0:c0 + nch])
                rdb = rdm[:, 0:nch].rearrange(
                    "p (a b) -> p a b", b=1).broadcast_to([128, nch, C])
                if orient == 0:
                    nc.vector.tensor_mul(z0s[:, c0:c0 + nch, :],
                                         pv[:, 0:nch, 0:C], rdb)
                else:
                    zb = zbuf[bi % 2]
                    zt = ztb[bi % 2]
                    z1 = smalls.tile([128, 4, C], BF16, tag="z1")
                    nc.vector.tensor_mul(z1[:, 0:nch, :],
                                         pv[:, 0:nch, 0:C], rdb)
                    nc.vector.tensor_add(zb[:, 0:nch, 0:C], z1[:, 0:nch, :],
                                         z0s[:, c0:c0 + nch, :])
                    nc.sync.dma_start_transpose(
                        out=zt[:, 0:nch, :],
                        in_=zb[:, 0:nch, :].rearrange("p a b -> p (a b)"))
                    nc.vector.tensor_add(
                        xpad[:, 2 * c0:2 * c0 + 2 * nch, 1:65],
                        zt[0:C, 0:nch, :].rearrange(
                            "p a (r w) -> p (a r) w", w=W_IMG),
                        eqh_sb[:, c0 * 128:c0 * 128 + nbl].rearrange(
                            "p (r w) -> p r w", w=W_IMG))
            return finish

        # ---- conv row-block ----
        def emit_conv(rb):
            ypt = ps_sp.tile([128, 1, 512], F32, tag="sp")
            yp = ypt[0:C, 0, :]
            for tap in range(9):
                dy, dx = tap // 3, tap % 3
                nc.tensor.matmul(
                    yp,
                    wconv_sb[:, tap, :],
                    xpad[:, 8 * rb + 1 + dy:8 * rb + 9 + dy, dx:dx + 64],
                    start=(tap == 0), stop=(tap == 8))
            nc.vector.bn_stats(out=st[:, rb, :], in_=yp)
            nc.vector.tensor_copy(y_sb[:, rb * 512:(rb + 1) * 512], yp)

        # ---- phase A (orientation 0) ----
        emit_el(1)            # el_l block 0 gates the first scores
        fin = None
        for bi in range(5):
            fin = run_block(bi, 0, hooks=[fin] if fin else [])
        fin()
        emit_el(13)           # any stragglers

        # ---- phase B (orientation 1) + interleaved conv ----
        ext_q = []
        if DIAG:
            ext_q = [emit_ext(0)]
        fin = None
        for bi in range(5):
            if DIAG and bi + 1 < 5:
                ext_q.append(emit_ext(bi + 1))
            hooks = []
            if fin is not None:
                hooks.append(fin)
            if bi >= 2:
                hooks.extend([None] * 6)
                hooks.append(lambda rb=bi - 2: emit_conv(rb))
            fin = run_block(bi, 1, exts=ext_q.pop(0) if DIAG else None,
                            hooks=hooks)
        fin()
        emit_conv(2)
        emit_conv(3)

        mv = smalls.tile([C, 2], F32, tag="mv")
        nc.vector.bn_aggr(out=mv[:], in_=st[:])

        # ---- BN stats AllGather + local reduce ----
        ccs = smalls.tile([C, 2], F32, tag="ccs")
        nc.vector.tensor_copy(ccs[:, 0:1], mv[:, 0:1])
        nc.vector.scalar_tensor_tensor(
            out=ccs[:, 1:2], in0=mv[:, 0:1], scalar=mv[:, 0:1],
            in1=mv[:, 1:2], op0=mybir.AluOpType.mult,
            op1=mybir.AluOpType.add)
        cc_in = dram.tile([C, 2], F32)
        cc_out = dram.tile([N_CORES, C, 2], F32, addr_space="Shared")
        nc.sync.dma_start(out=cc_in[:], in_=ccs[:])
        nc.gpsimd.collective_compute(
            "AllGather", mybir.AluOpType.bypass,
            replica_groups=[list(range(N_CORES))],
            ins=[cc_in.opt()], outs=[cc_out.opt()])
        gath = smalls.tile([C, 2, N_CORES], F32, tag="gath")
        nc.sync.dma_start(out=gath[:],
                          in_=cc_out[:].rearrange("r c v -> c v r"))
        red = smalls.tile([C, 2], F32, tag="red")
        nc.vector.tensor_reduce(red[:], gath[:], axis=mybir.AxisListType.X,
                                op=mybir.AluOpType.add)

        # mu = red0/8 ; var = red1/8 - mu^2 ; rstd = exp(-0.5*ln(var+eps))
        nc.vector.tensor_scalar_mul(red[:], red[:], 1.0 / N_CORES)
        mu = red[:, 0:1]
        var = smalls.tile([C, 1], F32, tag="var")
        mu2 = smalls.tile([C, 1], F32, tag="mu2")
        nc.vector.tensor_mul(mu2[:], mu, mu)
        nc.vector.tensor_sub(var[:], red[:, 1:2], mu2[:])
        lnv = smalls.tile([C, 1], F32, tag="lnv")
        nc.scalar.activation(out=lnv[:], in_=var[:],
                             func=mybir.ActivationFunctionType.Ln,
                             bias=eps_sb[:])
        rstd = smalls.tile([C, 1], F32, tag="rstd")
        nc.scalar.activation(out=rstd[:], in_=lnv[:],
                             func=mybir.ActivationFunctionType.Exp, scale=-0.5)
        scale_f = smalls.tile([C, 1], F32, tag="scale_f")
        bias_f = smalls.tile([C, 1], F32, tag="bias_f")
        nc.vector.tensor_mul(scale_f[:], gamma_sb[:], rstd[:])
        nc.vector.tensor_mul(bias_f[:], mu, scale_f[:])
        nc.vector.tensor_sub(bias_f[:], beta_sb[:], bias_f[:])

        # ---- BN + leaky relu (Prelu), write out in 4 chunks ----
        for hb in range(4):
            sl = slice(hb * 512, (hb + 1) * 512)
            osb = smalls.tile([C, 512], F32, tag="osb")
            nc.scalar.activation(out=osb[:], in_=y_sb[:, sl],
                                 func=mybir.ActivationFunctionType.Prelu,
                                 bias=bias_f[:], scale=scale_f[:],
                                 alpha=alpha_sb[:])
            nc.sync.dma_start(out=d_out[:, sl], in_=osb[:])

    nc.compile()
    return nc


def _get_program():
    global _COMPILED
    if _COMPILED is None:
        _COMPILED = _build_program()
    return _COMPILED


def _make_in_maps(exemplar, query, W_lin, W_conv, gamma, beta):
    E = np.asarray(exemplar, dtype=np.float32).reshape(4, C, HW)
    Q = np.asarray(query, dtype=np.float32).reshape(4, C, HW)
    wt = np.ascontiguousarray(np.asarray(W_lin, np.float32).T)
    wconv = np.ascontiguousarray(
        np.asarray(W_conv, np.float32).transpose(1, 2, 3, 0).reshape(C, 9 * C))
    g = np.asarray(gamma, np.float32).reshape(C, 1)
    b = np.asarray(beta, np.float32).reshape(C, 1)

    in_maps = []
    for k in range(N_CORES):
        s, h = divmod(k, 2)
        own = slice(2048 * h, 2048 * h + 2048)
        oth = slice(2048 * (1 - h), 2048 * (1 - h) + 2048)
        xq_p = np.concatenate([Q[s][:, own], Q[s][:, oth]], axis=1)
        xe_p = np.concatenate([E[s][:, own], E[s][:, oth]], axis=1)

        # l layout: 36 rows; l-row r -> sample row 32h - 2 + r (r=1..34)
        ehl = np.zeros((C, L), np.float32)
        qhl = np.zeros((C, L), np.float32)
        maskl = np.zeros((128, LCH), np.float32)
        for r in range(36):
            sr = 32 * h - 2 + r
            if 1 <= r <= 34 and 0 <= sr < 64:
                ehl[:, 64 * r:64 * r + 64] = E[s][:, 64 * sr:64 * sr + 64]
                qhl[:, 64 * r:64 * r + 64] = Q[s][:, 64 * sr:64 * sr + 64]
                kk, half = divmod(r, 2)
                maskl[64 * half:64 * half + 64, kk] = 1.0
        pack = np.concatenate([
            wt, ehl, xq_p, qhl, xe_p, (ehl + qhl), wconv], axis=1
        ).astype(NPBF16)
        in_maps.append({
            "pack": np.ascontiguousarray(pack),
            "xq": np.ascontiguousarray(xq_p.astype(NPBF16)),
            "xe": np.ascontiguousarray(xe_p.astype(NPBF16)),
            "maskl": np.ascontiguousarray(maskl.astype(NPBF16)),
            "gb": np.ascontiguousarray(np.concatenate([g, b], axis=1)),
        })
    return in_maps


def kernel(exemplar, query, W_lin, W_conv, gamma, beta):
    nc = _get_program()
    in_maps = _make_in_maps(exemplar, query, W_lin, W_conv, gamma, beta)
    res = bass_utils.run_bass_kernel_spmd(
        nc, in_maps, core_ids=list(range(N_CORES)), trace=False)
    out = np.empty((4, C, 64, 64), np.float32)
    for k in range(N_CORES):
        s, h = divmod(k, 2)
        out[s, :, 32 * h:32 * h + 32, :] = \
            res.results[k]["out"].reshape(C, 32, 64)
    return out
